# revision 14
# baseline (speedup 1.0000x reference)
"""Multi-head attention (B=2, S=2048, D=1024, H=16, causal + rel-pos-bias + RoPE)
on 8 Trainium2 NeuronCores.

Sharding: core c handles batch c//4 and head-group c%4 (4 heads = 256 model dims).
Each core computes its heads' Q/K/V projections (column-sharded weights), RoPE,
causal attention with relative position bias, and a partial output projection
(row-sharded Wo). Host sums the 4 partials per batch and adds Wo_b.

v3: PE warmup (DVFS ramp) + fine-grained startup DMA interleave so the first
matmul issues ~2us in; eb/x-wave-1 DMA streams interleaved per-task so neither
starves; split PSUM pools (proj/scores/ctx) for deeper pipelining; rope via
scalar-engine PSUM cast + out-shifted DVE multiplies; reciprocal on the [1,S]
denominator row before the PE broadcast; scores pairs interleaved with the
previous task's AV chunks.
"""

import math

import numpy as np
import ml_dtypes

import concourse.bass as bass
import concourse.mybir as mybir
import concourse.tile as tile
from concourse import bacc
from concourse.bass_utils import run_bass_kernel_spmd

BF16 = ml_dtypes.bfloat16

B, S, D, H = 2, 2048, 1024, 16
DK = 64
SCALE = math.sqrt(DK)
HPC = 4          # heads per core
GDIM = HPC * DK  # 256 model dims per core
N_CORES = 8
KT = S // 128    # 16 k-tiles
QC = S // 512    # 4 q-chunks
NWARM = 14       # PE warmup matmuls (DVFS ramp ~3us)

f32 = mybir.dt.float32
bf16 = mybir.dt.bfloat16


def _quads(qc):
    """kt quad-groups for one (h, qc) chunk: list of [(kt,n,q0)...] (len<=4)."""
    kts = list(range(4 * qc + 4))
    out = []
    for i in range(0, len(kts), 4):
        grp = []
        for kt in kts[i:i + 4]:
            if kt // 4 == qc:
                n = 512 - 128 * (kt % 4)
                q0 = 128 * kt
            else:
                n = 512
                q0 = 512 * qc
            grp.append((kt, n, q0))
        out.append(grp)
    return out


def _sched():
    for h in range(HPC):
        for qc in range(QC):
            for grp in _quads(qc):
                for kt, n, q0 in grp:
                    yield h, qc, kt, n, q0


EB_PER_HEAD = sum(128 * n for h, qc, kt, n, q0 in _sched()) // HPC
EB_TOTAL = EB_PER_HEAD * HPC

_PROGRAM = None


def _build_program():
    nc = bacc.Bacc("TRN2", target_bir_lowering=False, debug=False)

    dqT = nc.dram_tensor("qT", [8, 128, S], bf16, kind="ExternalInput").ap()
    dkT = nc.dram_tensor("kT", [8, 128, S], bf16, kind="ExternalInput").ap()
    dvT = nc.dram_tensor("vT", [8, 128, S], bf16, kind="ExternalInput").ap()
    dwq = nc.dram_tensor("wq", [8, 128, GDIM], bf16, kind="ExternalInput").ap()
    dwk = nc.dram_tensor("wk", [8, 128, GDIM], bf16, kind="ExternalInput").ap()
    dwv = nc.dram_tensor("wv", [8, 128, GDIM], bf16, kind="ExternalInput").ap()
    dwo = nc.dram_tensor("wo", [2, 128, D], bf16, kind="ExternalInput").ap()
    deb = nc.dram_tensor("eb", [EB_TOTAL], bf16, kind="ExternalInput").ap()
    dcos = nc.dram_tensor("cosT", [128, S], bf16, kind="ExternalInput").ap()
    dssin = nc.dram_tensor("ssinT", [128, S], bf16, kind="ExternalInput").ap()
    dout = nc.dram_tensor("out", [S, D], bf16, kind="ExternalOutput").ap()

    # per-head packed-bias offset of each qc block
    woff_qc = []
    acc = 0
    for qc in range(QC):
        woff_qc.append(acc)
        for grp in _quads(qc):
            acc += 128 * sum(n for kt, n, q0 in grp)
    assert acc == EB_PER_HEAD

    with tile.TileContext(nc) as tc:
        with tc.tile_pool(name="consts", bufs=1) as consts, \
             tc.tile_pool(name="persist", bufs=1) as persist, \
             tc.tile_pool(name="xvp", bufs=1) as xvp, \
             tc.tile_pool(name="ropep", bufs=1) as ropep, \
             tc.tile_pool(name="attn_sb", bufs=1) as attn_sb, \
             tc.tile_pool(name="normp", bufs=1) as normp, \
             tc.tile_pool(name="outst", bufs=4) as outst, \
             tc.tile_pool(name="psum", bufs=1, space="PSUM") as psum:

            # ---- constants & persistent tiles ----
            wq_s = consts.tile([128, 8, GDIM], bf16)
            wk_s = consts.tile([128, 8, GDIM], bf16)
            wv_s = consts.tile([128, 8, GDIM], bf16)
            wo_s = consts.tile([128, 2, D], bf16)
            cos_s = consts.tile([128, S], bf16)
            ssin_s = consts.tile([128, S], bf16)
            warm = consts.tile([128, 512], bf16)

            QT = [persist.tile([128, S], bf16, name=f"QT{m}") for m in range(2)]
            KTt = [persist.tile([128, S], bf16, name=f"KTt{m}") for m in range(2)]
            Vt = persist.tile([128, KT, HPC, DK + 1], bf16)
            cxT = [persist.tile([128, S], bf16, name=f"cxT{m}") for m in range(2)]
            nc.vector.memset(Vt[:, :, :, DK:DK + 1], 1.0)
            nc.vector.memset(warm, 0.0)

            # ones row for the denominator broadcast matmuls
            ones_b = consts.tile([1, 64], bf16)
            nc.vector.memset(ones_b, 1.0)

            # ---- PE warmup: ramp DVFS while startup DMAs land ----
            wps = psum.tile([128, 512], f32, tag="ps", bufs=3, name="wps")
            for _ in range(NWARM):
                nc.tensor.matmul(wps, lhsT=warm[:, 0:128], rhs=warm,
                                 start=True, stop=True)
            nc.vector.tensor_copy(out=warm, in_=wps)

            # ---- startup DMAs, finely interleaved per consumption order ----
            xq0 = xvp.tile([128, 8, 1024], bf16, tag="xv", bufs=4, name="xq0")
            for t in range(8):
                nc.sync.dma_start(out=wq_s[:, t, :], in_=dwq[t])
                nc.sync.dma_start(out=xq0[:, t, :], in_=dqT[t][:, 0:1024])
            xk0 = xvp.tile([128, 8, 1024], bf16, tag="xv", bufs=4, name="xk0")
            for t in range(8):
                nc.gpsimd.dma_start(out=wk_s[:, t, :], in_=dwk[t])
                nc.gpsimd.dma_start(out=xk0[:, t, :], in_=dkT[t][:, 0:1024])
            for t in range(2):
                nc.gpsimd.dma_start(out=wo_s[:, t, :], in_=dwo[t])
            nc.scalar.dma_start(out=cos_s, in_=dcos)
            nc.scalar.dma_start(out=ssin_s, in_=dssin)
            vw0 = xvp.tile([128, 8, 1024], bf16, tag="xv", bufs=4, name="vw0")
            for t in range(8):
                nc.scalar.dma_start(out=wv_s[:, t, :], in_=dwv[t])
            for h in range(2):
                for t in range(8):
                    nc.scalar.dma_start(
                        out=vw0[:, t, 512 * h:512 * h + 512],
                        in_=dvT[t][:, 512 * h:512 * h + 512])

            # ---- helpers ----
            def rope_pair(pp2, dst, w, swap_dma):
                """pp2: psum [128,1024] raw proj.T for cols 1024w..1024w+1024."""
                ppsb = ropep.tile([128, 1024], bf16, tag="ppsb", bufs=2,
                                  name="ppsb")
                for j in range(2):
                    nc.scalar.copy(out=ppsb[:, 512 * j:512 * j + 512],
                                   in_=pp2[:, 512 * j:512 * j + 512])
                # rotate-half block swap: SBUF->SBUF DMAs on the scalar ring
                # when it is idle (wave 1), DVE copies otherwise (wave 0)
                psw = ropep.tile([128, 1024], bf16, tag="psw", bufs=2,
                                 name="psw")
                for o, i in ((0, 32), (32, 0), (64, 96), (96, 64)):
                    if swap_dma:
                        nc.scalar.dma_start(out=psw[o:o + 32, :],
                                            in_=ppsb[i:i + 32, :])
                    else:
                        nc.vector.tensor_copy(out=psw[o:o + 32, :],
                                              in_=ppsb[i:i + 32, :])
                for j in range(2):
                    cs = slice(512 * (2 * w + j), 512 * (2 * w + j) + 512)
                    js = slice(512 * j, 512 * j + 512)
                    tcos = ropep.tile([128, 512], bf16, tag="tcos", bufs=3,
                                      name="tcos")
                    nc.vector.tensor_mul(out=tcos, in0=ppsb[:, js],
                                         in1=cos_s[:, cs])
                    tsin = ropep.tile([128, 512], bf16, tag="tsin", bufs=3,
                                      name="tsin")
                    nc.vector.tensor_mul(out=tsin, in0=psw[:, js],
                                         in1=ssin_s[:, cs])
                    nc.gpsimd.tensor_add(out=dst[:, cs], in0=tcos, in1=tsin)

            def qk_proj(m, w, xw, wsrc, dst):
                pp2 = psum.tile([128, 1024], f32, tag="ps", bufs=3,
                                name="pp2")
                for j in range(2):
                    for t in range(8):
                        nc.tensor.matmul(
                            pp2[:, 512 * j:512 * j + 512],
                            lhsT=wsrc[:, t, 128 * m:128 * m + 128],
                            rhs=xw[:, t, 512 * j:512 * j + 512],
                            start=(t == 0), stop=(t == 7))
                rope_pair(pp2, dst, w, swap_dma=(w == 1))

            def v_proj_tile(w, vww, tt):
                pv = psum.tile([128, GDIM], f32, tag="ps", bufs=3, name="pv")
                lo = 128 * (tt - 8 * w)
                for t in range(8):
                    nc.tensor.matmul(
                        pv, lhsT=vww[:, t, lo:lo + 128], rhs=wv_s[:, t, :],
                        start=(t == 0), stop=(t == 7))
                nc.scalar.copy(
                    out=Vt[:, tt, :, 0:DK],
                    in_=pv.rearrange("p (h d) -> p h d", h=HPC))

            # ---- attention task list ----
            tasks = []
            for qc in range(QC):
                for m in range(2):
                    qs = _quads(qc)
                    woff = woff_qc[qc]
                    for qi, grp in enumerate(qs):
                        tasks.append(dict(
                            qc=qc, m=m, grp=grp, woff=woff,
                            first=(qi == 0), last=(qi == len(qs) - 1)))
                        woff += 128 * sum(n for kt, n, q0 in grp)

            eb_emitted = [False] * len(tasks)
            eb_q = [nc.sync, nc.gpsimd]

            def ensure_eb(i):
                if i >= len(tasks) or eb_emitted[i]:
                    return
                t = tasks[i]
                gn = sum(n for kt, n, q0 in t['grp'])
                ebt = []
                for a in range(2):
                    e = attn_sb.tile([128, gn], bf16, tag=f"ebt{a}", bufs=4,
                                     name=f"ebt{a}")
                    base = (2 * t['m'] + a) * EB_PER_HEAD + t['woff']
                    eb_q[(i + a) % 2].dma_start(
                        out=e,
                        in_=deb[base:base + 128 * gn].rearrange(
                            "(p n) -> p n", p=128))
                    ebt.append(e)
                t['ebt'] = ebt
                eb_emitted[i] = True

            def emit_scores_praw(t):
                gn = sum(n for kt, n, q0 in t['grp'])
                t['praw'] = [attn_sb.tile([128, gn], bf16, tag=f"praw{a}",
                                          bufs=3, name=f"praw{a}")
                             for a in range(2)]
                t['goffs'] = []
                goff = 0
                for kt, n, q0 in t['grp']:
                    t['goffs'].append(goff)
                    goff += n

            def emit_scores_pair(t, pi):
                m, grp = t['m'], t['grp']
                pair = grp[2 * pi:2 * pi + 2]
                if not pair:
                    return
                pn = sum(n for kt, n, q0 in pair)
                goff = t['goffs'][2 * pi]
                pss = [psum.tile([128, pn], f32, tag="ps", bufs=3,
                                 name=f"pss{a}") for a in range(2)]
                for a in range(2):
                    soff = 0
                    for kt, n, q0 in pair:
                        nc.tensor.matmul(
                            pss[a][:, soff:soff + n],
                            lhsT=KTt[m][64 * a:64 * a + DK,
                                        128 * kt:128 * kt + 128],
                            rhs=QT[m][64 * a:64 * a + DK, q0:q0 + n],
                            start=True, stop=True,
                            tile_position=(64 * a, 0))
                        soff += n
                for a in range(2):
                    nc.scalar.activation(
                        out=t['praw'][a][:, goff:goff + pn], in_=pss[a],
                        func=mybir.ActivationFunctionType.Exp)

            def emit_praw_mul(t):
                for a in range(2):
                    nc.vector.tensor_mul(out=t['praw'][a], in0=t['praw'][a],
                                         in1=t['ebt'][a])

            def emit_av_chunk(t, pi):
                qc, m, grp = t['qc'], t['m'], t['grp']
                chunk = grp[2 * pi:2 * pi + 2]
                if not chunk:
                    return
                if t['first'] and pi == 0:
                    t['pcx_grp'] = [psum.tile([DK + 1, 512], f32, tag="pcx",
                                              bufs=2, name=f"pcx{a}")
                                    for a in range(2)]
                    pcx_cur[0] = t['pcx_grp']
                pcx = pcx_cur[0]
                last_kt = 4 * qc + 3
                for ci, (kt, n, q0) in enumerate(chunk):
                    goff = t['goffs'][2 * pi + ci]
                    co = q0 - 512 * qc
                    for a in range(2):
                        nc.tensor.matmul(
                            pcx[a][:, co:co + n],
                            lhsT=Vt[:, kt, 2 * m + a, :],
                            rhs=t['praw'][a][:, goff:goff + n],
                            start=(kt == 0), stop=(kt == last_kt))

            pcx_cur = [None]

            def emit_norm(qc, m, pcx):
                lr2 = normp.tile([1, 1024], bf16, tag="lr2", bufs=2,
                                 name="lr2")
                for a in range(2):
                    nc.vector.tensor_copy(out=lr2[0:1, 512 * a:512 * a + 512],
                                          in_=pcx[a][DK:DK + 1, :])
                for a in range(2):
                    pb = psum.tile([DK, 512], f32, tag="ps", bufs=3,
                                   name="pb")
                    nc.tensor.matmul(pb, lhsT=ones_b,
                                     rhs=lr2[0:1, 512 * a:512 * a + 512],
                                     start=True, stop=True)
                    crec = normp.tile([DK, 512], f32, tag=f"crec{a}", bufs=2,
                                      name=f"crec{a}")
                    nc.vector.reciprocal_approx_fast(out=crec, in_=pb)
                    nc.vector.tensor_mul(
                        out=cxT[m][64 * a:64 * a + DK,
                                   512 * qc:512 * qc + 512],
                        in0=pcx[a][0:DK, :], in1=crec)

            out_q = [nc.sync, nc.gpsimd]

            def emit_outproj(qc):
                for tt in range(4 * qc, 4 * qc + 4):
                    for e in range(2):
                        po = psum.tile([128, 512], f32, tag="ps", bufs=3,
                                       name="po")
                        for m in range(2):
                            nc.tensor.matmul(
                                po,
                                lhsT=cxT[m][:, 128 * tt:128 * tt + 128],
                                rhs=wo_s[:, m, 512 * e:512 * e + 512],
                                start=(m == 0), stop=(m == 1))
                        ost = outst.tile([128, 512], bf16, tag="ost",
                                         name="ost")
                        nc.vector.tensor_copy(out=ost, in_=po)
                        out_q[(2 * tt + e) % 2].dma_start(
                            out=dout[128 * tt:128 * tt + 128,
                                     512 * e:512 * e + 512],
                            in_=ost)

            def run_phase(ph_tasks, base_idx, prefetch):
                """prefetch: list of callables; one popped after each task."""
                prev = None
                for k, t in enumerate(ph_tasks):
                    for d in range(4):
                        ensure_eb(base_idx + k + d)
                    emit_scores_praw(t)
                    npair = (len(t['grp']) + 1) // 2
                    nav = (len(prev['grp']) + 1) // 2 if prev else 0
                    for pi in range(max(npair, nav)):
                        if pi < npair:
                            emit_scores_pair(t, pi)
                        if prev is not None and pi < nav:
                            emit_av_chunk(prev, pi)
                    emit_praw_mul(t)
                    if prev is not None and prev['last']:
                        emit_norm(prev['qc'], prev['m'], pcx_cur[0])
                        if prev['m'] == 1:
                            emit_outproj(prev['qc'])
                    for _ in range(2):
                        if prefetch:
                            prefetch.pop(0)()
                    prev = t
                for pi in range((len(prev['grp']) + 1) // 2):
                    emit_av_chunk(prev, pi)
                if prev['last']:
                    emit_norm(prev['qc'], prev['m'], pcx_cur[0])
                    if prev['m'] == 1:
                        emit_outproj(prev['qc'])

            # ---- wave 0: projections for q/k columns 0..1023, V tiles 0..7
            for m in range(2):
                qk_proj(m, 0, xq0, wq_s, QT[m])
            for m in range(2):
                qk_proj(m, 0, xk0, wk_s, KTt[m])
            for tt in range(8):
                v_proj_tile(0, vw0, tt)

            # wave-1 x prefetch: interleaved with ph0's eb stream per-task
            xq1 = xvp.tile([128, 8, 1024], bf16, tag="xv", bufs=4, name="xq1")
            xk1 = xvp.tile([128, 8, 1024], bf16, tag="xv", bufs=4, name="xk1")
            vw1 = xvp.tile([128, 8, 1024], bf16, tag="xv", bufs=4, name="vw1")
            prefetch = []
            for t in range(8):
                prefetch.append(lambda t=t: nc.sync.dma_start(
                    out=xq1[:, t, :], in_=dqT[t][:, 1024:2048]))
            for t in range(8):
                prefetch.append(lambda t=t: nc.gpsimd.dma_start(
                    out=xk1[:, t, :], in_=dkT[t][:, 1024:2048]))
            for t in range(8):
                prefetch.append(lambda t=t: nc.sync.dma_start(
                    out=vw1[:, t, 0:512], in_=dvT[t][:, 1024:1536]))

            # ---- attention qc0 + qc1 ----
            ph0 = [t for t in tasks if t['qc'] < 2]
            run_phase(ph0, 0, prefetch)

            # rest of vw1 + ph1 eb head-start before wave-1 proj
            for fn in prefetch:
                fn()
            prefetch = []
            for t in range(8):
                nc.gpsimd.dma_start(out=vw1[:, t, 512:1024],
                                    in_=dvT[t][:, 1536:2048])
            for d in range(3):
                ensure_eb(len(ph0) + d)

            # ---- wave 1: projections for q/k columns 1024..2047, V tiles 8..15
            for m in range(2):
                qk_proj(m, 1, xq1, wq_s, QT[m])
            for m in range(2):
                qk_proj(m, 1, xk1, wk_s, KTt[m])
            for tt in range(8, 16):
                v_proj_tile(1, vw1, tt)

            # ---- attention qc2 + qc3 ----
            ph1 = [t for t in tasks if t['qc'] >= 2]
            run_phase(ph1, len(ph0), [])

    nc.compile()
    return nc


def _get_program():
    global _PROGRAM
    if _PROGRAM is None:
        _PROGRAM = _build_program()
    return _PROGRAM


def _rope_tables():
    half = DK // 2
    inv_freq = 1.0 / (10000.0 ** (np.arange(half, dtype=np.float64) / half))
    ang = np.arange(S, dtype=np.float64)[:, None] * inv_freq[None, :]  # [S, 32]
    cos = np.cos(ang).T  # [32, S]
    sin = np.sin(ang).T
    cos64 = np.concatenate([cos, cos], axis=0)            # [64, S]
    # swapped-first rope: psw[0:32] = x2, psw[32:64] = x1 (DMA block swap);
    # dst[0:32] = x1 cos + psw[0:32] * (-sin) = x1 cos - x2 sin
    # dst[32:64] = x2 cos + psw[32:64] * (+sin) = x2 cos + x1 sin
    ssin64 = np.concatenate([-sin, sin], axis=0)
    cosT = np.tile(cos64, (2, 1)).astype(BF16)            # [128, S]
    ssinT = np.tile(ssin64, (2, 1)).astype(BF16)
    return np.ascontiguousarray(cosT), np.ascontiguousarray(ssinT)


def _pack_ebias(bias_g):
    """bias_g: [HPC, S, S] f32 (this group's heads). Returns packed 1D bf16,
    one contiguous [128, gn] row-major block per kt-quad (matching the wide
    SBUF tiles the kernel DMAs)."""
    out = np.empty(EB_TOTAL, dtype=BF16)
    off = 0
    tri = np.triu(np.ones((128, 128), dtype=np.float32))
    for h in range(HPC):
        for qc in range(QC):
            for grp in _quads(qc):
                blks = []
                for kt, n, q0 in grp:
                    blk = np.exp(
                        bias_g[h, q0:q0 + n, 128 * kt:128 * kt + 128]
                        .astype(np.float64)).T.astype(np.float32)  # [128, n]
                    if kt // 4 == qc:
                        blk[:, 0:128] *= tri
                    blks.append(blk)
                wide = np.concatenate(blks, axis=1)  # [128, gn]
                gn = wide.shape[1]
                out[off:off + 128 * gn] = wide.astype(BF16).reshape(-1)
                off += 128 * gn
    assert off == EB_TOTAL
    return out


def _prep_inputs(query, key, value, rel_pos_bias, Wq, Wk, Wv, Wo_w):
    cosT, ssinT = _rope_tables()
    xT = {}
    for nm, x in (("q", query), ("k", key), ("v", value)):
        for b in range(B):
            t = np.ascontiguousarray(x[b].T.reshape(8, 128, S)).astype(BF16)
            xT[(nm, b)] = t
    wqs, wks, wvs, wos, ebs = {}, {}, {}, {}, {}
    for g in range(4):
        sl = slice(GDIM * g, GDIM * (g + 1))
        wqs[g] = np.ascontiguousarray(
            (Wq[sl, :] / SCALE).T.reshape(8, 128, GDIM)).astype(BF16)
        wks[g] = np.ascontiguousarray(Wk[sl, :].T.reshape(8, 128, GDIM)).astype(BF16)
        wvs[g] = np.ascontiguousarray(Wv[sl, :].T.reshape(8, 128, GDIM)).astype(BF16)
        wos[g] = np.ascontiguousarray(Wo_w[:, sl].T.reshape(2, 128, D)).astype(BF16)
        ebs[g] = _pack_ebias(rel_pos_bias[0, HPC * g:HPC * (g + 1)])
    in_maps = []
    for c in range(N_CORES):
        b, g = c // 4, c % 4
        in_maps.append({
            "qT": xT[("q", b)], "kT": xT[("k", b)], "vT": xT[("v", b)],
            "wq": wqs[g], "wk": wks[g], "wv": wvs[g], "wo": wos[g],
            "eb": ebs[g], "cosT": cosT, "ssinT": ssinT,
        })
    return in_maps


def _run(query, key, value, rel_pos_bias, Wq, Wk, Wv, Wo_w, Wo_b, trace=False,
         **trace_kwargs):
    nc = _get_program()
    in_maps = _prep_inputs(query, key, value, rel_pos_bias, Wq, Wk, Wv, Wo_w)
    res = run_bass_kernel_spmd(nc, in_maps, core_ids=list(range(N_CORES)),
                               trace=trace, **trace_kwargs)
    out = np.empty((B, S, D), dtype=np.float32)
    for b in range(B):
        acc = res.results[4 * b]["out"].astype(np.float32)
        for g in range(1, 4):
            acc = acc + res.results[4 * b + g]["out"].astype(np.float32)
        out[b] = acc + Wo_b[None, :]
    return out, res


def _cpu_fallback(query, key, value, mask, rel_pos_bias, Wq, Wk, Wv, Wo_w, Wo_b):
    def rope_np(x):
        half = DK // 2
        inv_freq = 1.0 / (10000.0 ** (np.arange(half, dtype=np.float32) / half))
        ang = np.arange(S, dtype=np.float32)[:, None] * inv_freq[None, :]
        cos = np.concatenate([np.cos(ang), np.cos(ang)], axis=-1)[None, None]
        sin = np.concatenate([np.sin(ang), np.sin(ang)], axis=-1)[None, None]
        x1, x2 = x[..., :half], x[..., half:]
        rot = np.concatenate([-x2, x1], axis=-1)
        return x * cos + rot * sin

    q = np.einsum('bsd,ed->bse', query, Wq).reshape(B, S, H, DK).transpose(0, 2, 1, 3)
    k = np.einsum('bsd,ed->bse', key, Wk).reshape(B, S, H, DK).transpose(0, 2, 1, 3)
    v = np.einsum('bsd,ed->bse', value, Wv).reshape(B, S, H, DK).transpose(0, 2, 1, 3)
    q, k = rope_np(q), rope_np(k)
    sc = np.einsum('bhqd,bhkd->bhqk', q, k) / SCALE + rel_pos_bias
    sc = np.where(mask, sc, -np.inf)
    sc = sc - sc.max(axis=-1, keepdims=True)
    e = np.exp(sc)
    attn = e / e.sum(axis=-1, keepdims=True)
    ctx = np.einsum('bhqk,bhkd->bhqd', attn, v)
    ctx = ctx.transpose(0, 2, 1, 3).reshape(B, S, D)
    return (np.einsum('bsd,ed->bse', ctx, Wo_w) + Wo_b).astype(np.float32)


def kernel(query, key, value, mask, rel_pos_bias, Wq, Wk, Wv, Wo_w, Wo_b):
    query = np.asarray(query, dtype=np.float32)
    key = np.asarray(key, dtype=np.float32)
    value = np.asarray(value, dtype=np.float32)
    mask = np.asarray(mask)
    rel_pos_bias = np.asarray(rel_pos_bias, dtype=np.float32)
    Wq = np.asarray(Wq, dtype=np.float32)
    Wk = np.asarray(Wk, dtype=np.float32)
    Wv = np.asarray(Wv, dtype=np.float32)
    Wo_w = np.asarray(Wo_w, dtype=np.float32)
    Wo_b = np.asarray(Wo_b, dtype=np.float32)

    if not np.array_equal(mask.reshape(S, S),
                          np.tril(np.ones((S, S), dtype=bool))):
        return _cpu_fallback(query, key, value, mask, rel_pos_bias,
                             Wq, Wk, Wv, Wo_w, Wo_b)

    out, _ = _run(query, key, value, rel_pos_bias, Wq, Wk, Wv, Wo_w, Wo_b)
    return out


# revision 15
# speedup vs baseline: 1.0406x; 1.0406x over previous
"""Multi-head attention (B=2, S=2048, D=1024, H=16, causal + rel-pos-bias + RoPE)
on 8 Trainium2 NeuronCores.

Sharding: core c handles batch c//4 and head-group c%4 (4 heads = 256 model dims).
Each core computes its heads' Q/K/V projections (column-sharded weights), RoPE,
causal attention with relative position bias, and a partial output projection
(row-sharded Wo). Host sums the 4 partials per batch and adds Wo_b.

v2: wave-split projections interleaved with attention (qc0/qc1 run between the
two projection waves), RoPE without scalar-engine copies (sign-baked sin table
+ partition-block-swap copies on DVE), batched normalization, bf16 output,
deeper PSUM pipelining, prioritized DMA queues.
"""

import math

import numpy as np
import ml_dtypes

import concourse.bass as bass
import concourse.mybir as mybir
import concourse.tile as tile
from concourse import bacc
from concourse.bass_utils import run_bass_kernel_spmd

BF16 = ml_dtypes.bfloat16

B, S, D, H = 2, 2048, 1024, 16
DK = 64
SCALE = math.sqrt(DK)
HPC = 4          # heads per core
GDIM = HPC * DK  # 256 model dims per core
N_CORES = 8
KT = S // 128    # 16 k-tiles
QC = S // 512    # 4 q-chunks

f32 = mybir.dt.float32
f32r = mybir.dt.float32r
bf16 = mybir.dt.bfloat16


def _quads(qc):
    """kt quad-groups for one (h, qc) chunk: list of [(kt,n,q0)...] (len<=4)."""
    kts = list(range(4 * qc + 4))
    out = []
    for i in range(0, len(kts), 4):
        grp = []
        for kt in kts[i:i + 4]:
            if kt // 4 == qc:
                n = 512 - 128 * (kt % 4)
                q0 = 128 * kt
            else:
                n = 512
                q0 = 512 * qc
            grp.append((kt, n, q0))
        out.append(grp)
    return out


def _sched():
    for h in range(HPC):
        for qc in range(QC):
            for grp in _quads(qc):
                for kt, n, q0 in grp:
                    yield h, qc, kt, n, q0


EB_PER_HEAD = sum(128 * n for h, qc, kt, n, q0 in _sched()) // HPC
EB_TOTAL = EB_PER_HEAD * HPC

_PROGRAM = None


def _build_program():
    nc = bacc.Bacc("TRN2", target_bir_lowering=False, debug=False)

    dqT = nc.dram_tensor("qT", [8, 128, S], bf16, kind="ExternalInput").ap()
    dkT = nc.dram_tensor("kT", [8, 128, S], bf16, kind="ExternalInput").ap()
    dvT = nc.dram_tensor("vT", [8, 128, S], bf16, kind="ExternalInput").ap()
    dwq = nc.dram_tensor("wq", [8, 128, GDIM], bf16, kind="ExternalInput").ap()
    dwk = nc.dram_tensor("wk", [8, 128, GDIM], bf16, kind="ExternalInput").ap()
    dwv = nc.dram_tensor("wv", [8, 128, GDIM], bf16, kind="ExternalInput").ap()
    dwo = nc.dram_tensor("wo", [2, 128, D], bf16, kind="ExternalInput").ap()
    deb = nc.dram_tensor("eb", [EB_TOTAL], bf16, kind="ExternalInput").ap()
    dcos = nc.dram_tensor("cosT", [128, S], bf16, kind="ExternalInput").ap()
    dssin = nc.dram_tensor("ssinT", [128, S], bf16, kind="ExternalInput").ap()
    dout = nc.dram_tensor("out", [S, D], bf16, kind="ExternalOutput").ap()

    # per-head packed-bias offset of each qc block
    woff_qc = []
    acc = 0
    for qc in range(QC):
        woff_qc.append(acc)
        for grp in _quads(qc):
            acc += 128 * sum(n for kt, n, q0 in grp)
    assert acc == EB_PER_HEAD

    with tile.TileContext(nc) as tc:
        with tc.tile_pool(name="consts", bufs=1) as consts, \
             tc.tile_pool(name="persist", bufs=1) as persist, \
             tc.tile_pool(name="xvp", bufs=1) as xvp, \
             tc.tile_pool(name="ropep", bufs=1) as ropep, \
             tc.tile_pool(name="attn_sb", bufs=1) as attn_sb, \
             tc.tile_pool(name="normp", bufs=1) as normp, \
             tc.tile_pool(name="outst", bufs=3) as outst, \
             tc.tile_pool(name="psum", bufs=1, space="PSUM") as psum:

            # ---- constants & persistent tiles ----
            warm = consts.tile([128, 512], bf16)
            wq_s = consts.tile([128, 8, GDIM], bf16)
            wk_s = consts.tile([128, 8, GDIM], bf16)
            wv_s = consts.tile([128, 8, GDIM], bf16)
            wo_s = consts.tile([128, 2, D], bf16)
            cos_s = consts.tile([128, S], bf16)
            ssin_s = consts.tile([128, S], bf16)

            QT = [persist.tile([128, S], bf16, name=f"QT{m}") for m in range(2)]
            KTt = [persist.tile([128, S], bf16, name=f"KTt{m}") for m in range(2)]
            Vt = persist.tile([128, KT, HPC, DK + 1], bf16)
            cxT = [persist.tile([128, S], bf16, name=f"cxT{m}") for m in range(2)]
            nc.vector.memset(Vt[:, :, :, DK:DK + 1], 1.0)
            nc.vector.memset(warm, 0.0)
            # PE warmup: ramp DVFS while startup DMAs land
            wps = psum.tile([128, 512], f32, tag="ps", bufs=3, name="wps")
            for _ in range(14):
                nc.tensor.matmul(wps, lhsT=warm[:, 0:128], rhs=warm,
                                 start=True, stop=True)
            nc.vector.tensor_copy(out=warm, in_=wps)

            # ones row for the denominator broadcast matmuls
            ones_b = consts.tile([1, 64], bf16)
            nc.vector.memset(ones_b, 1.0)

            # ---- input DMAs, prioritized ----
            # sync: wq + x-of-Q wave0 (first compute)
            xq0 = xvp.tile([128, 8, 1024], bf16, tag="xv", bufs=4, name="xq0")
            for t in range(8):
                nc.sync.dma_start(out=wq_s[:, t, :], in_=dwq[t])
                nc.sync.dma_start(out=xq0[:, t, :], in_=dqT[t][:, 0:1024])
            # gpsimd: wk + x-of-K wave0
            xk0 = xvp.tile([128, 8, 1024], bf16, tag="xv", bufs=4, name="xk0")
            for t in range(8):
                nc.gpsimd.dma_start(out=wk_s[:, t, :], in_=dwk[t])
                nc.gpsimd.dma_start(out=xk0[:, t, :], in_=dkT[t][:, 0:1024])
            # scalar: rope tables, wv, v wave0
            nc.scalar.dma_start(out=cos_s, in_=dcos)
            nc.scalar.dma_start(out=ssin_s, in_=dssin)
            vw0 = xvp.tile([128, 8, 1024], bf16, tag="xv", bufs=4, name="vw0")
            for t in range(8):
                nc.scalar.dma_start(out=wv_s[:, t, :], in_=dwv[t])
                nc.scalar.dma_start(out=vw0[:, t, :], in_=dvT[t][:, 0:1024])
            # gpsimd: wo (needed from first outproj, ~mid-kernel)
            for t in range(2):
                nc.gpsimd.dma_start(out=wo_s[:, t, :], in_=dwo[t])

            # ---- helpers ----
            def rope(pp_half, dst, n):
                """pp_half: psum [128,512] raw proj.T for 512-col chunk n."""
                t1 = ropep.tile([128, 512], bf16, tag="t1", bufs=3, name="t1")
                nc.vector.tensor_mul(out=t1, in0=pp_half,
                                     in1=ssin_s[:, 512 * n:512 * n + 512])
                # rotate-half: partition-block swap (out-shifted copies are
                # legal on DVE; TT inputs must share a partition base)
                t1s = ropep.tile([128, 512], bf16, tag="t1s", bufs=3,
                                 name="t1s")
                for o, i in ((0, 32), (32, 0), (64, 96), (96, 64)):
                    nc.vector.tensor_copy(out=t1s[o:o + 32, :],
                                          in_=t1[i:i + 32, :])
                ds = dst[:, 512 * n:512 * n + 512]
                nc.vector.tensor_mul(out=ds, in0=pp_half,
                                     in1=cos_s[:, 512 * n:512 * n + 512])
                nc.vector.tensor_add(out=ds, in0=ds, in1=t1s)

            def qk_proj(m, w, xw, wsrc, dst):
                pp = psum.tile([128, 1024], f32, tag="ps", bufs=3, name="pp")
                for t in range(8):
                    for j in range(2):
                        nc.tensor.matmul(
                            pp[:, 512 * j:512 * j + 512],
                            lhsT=wsrc[:, t, 128 * m:128 * m + 128],
                            rhs=xw[:, t, 512 * j:512 * j + 512],
                            start=(t == 0), stop=(t == 7))
                for j in range(2):
                    rope(pp[:, 512 * j:512 * j + 512], dst, 2 * w + j)

            def v_proj(w, vww):
                for tt in range(8 * w, 8 * w + 8):
                    pv = psum.tile([128, GDIM], f32, tag="ps", bufs=3,
                                   name="pv")
                    for t in range(8):
                        nc.tensor.matmul(
                            pv,
                            lhsT=vww[:, t,
                                     128 * (tt - 8 * w):128 * (tt - 8 * w) + 128],
                            rhs=wv_s[:, t, :],
                            start=(t == 0), stop=(t == 7))
                    nc.scalar.copy(
                        out=Vt[:, tt, :, 0:DK],
                        in_=pv.rearrange("p (h d) -> p h d", h=HPC))

            # ---- attention task list ----
            tasks = []
            for qc in range(QC):
                for m in range(2):
                    qs = _quads(qc)
                    woff = woff_qc[qc]
                    for qi, grp in enumerate(qs):
                        tasks.append(dict(
                            qc=qc, m=m, grp=grp, woff=woff,
                            first=(qi == 0), last=(qi == len(qs) - 1)))
                        woff += 128 * sum(n for kt, n, q0 in grp)

            eb_emitted = [False] * len(tasks)
            eb_q = [nc.sync, nc.gpsimd]

            def ensure_eb(i):
                if i >= len(tasks) or eb_emitted[i]:
                    return
                t = tasks[i]
                gn = sum(n for kt, n, q0 in t['grp'])
                ebt = []
                for a in range(2):
                    e = attn_sb.tile([128, gn], bf16, tag=f"ebt{a}", bufs=3,
                                     name=f"ebt{a}")
                    base = (2 * t['m'] + a) * EB_PER_HEAD + t['woff']
                    eb_q[(i + a) % 2].dma_start(
                        out=e,
                        in_=deb[base:base + 128 * gn].rearrange(
                            "(p n) -> p n", p=128))
                    ebt.append(e)
                t['ebt'] = ebt
                eb_emitted[i] = True

            pcx_cur = [None]

            def emit_scores(t):
                m, grp = t['m'], t['grp']
                gn = sum(n for kt, n, q0 in grp)
                praw = [attn_sb.tile([128, gn], bf16, tag=f"praw{a}", bufs=3,
                                     name=f"praw{a}") for a in range(2)]
                goff = 0
                for pi in range(0, len(grp), 2):
                    pair = grp[pi:pi + 2]
                    pn = sum(n for kt, n, q0 in pair)
                    pss = [psum.tile([128, pn], f32, tag="ps", bufs=3,
                                     name=f"pss{a}") for a in range(2)]
                    for a in range(2):
                        soff = 0
                        for kt, n, q0 in pair:
                            nc.tensor.matmul(
                                pss[a][:, soff:soff + n],
                                lhsT=KTt[m][64 * a:64 * a + DK,
                                            128 * kt:128 * kt + 128],
                                rhs=QT[m][64 * a:64 * a + DK, q0:q0 + n],
                                start=True, stop=True,
                                tile_position=(64 * a, 0))
                            soff += n
                    for a in range(2):
                        nc.scalar.activation(
                            out=praw[a][:, goff:goff + pn], in_=pss[a],
                            func=mybir.ActivationFunctionType.Exp)
                    goff += pn
                for a in range(2):
                    nc.vector.tensor_mul(out=praw[a], in0=praw[a],
                                         in1=t['ebt'][a])
                t['praw'] = praw

            def emit_av(t):
                qc, m, grp = t['qc'], t['m'], t['grp']
                if t['first']:
                    pcx_cur[0] = [psum.tile([DK + 1, 512], f32, tag="pcx",
                                            bufs=2, name=f"pcx{a}")
                                  for a in range(2)]
                pcx = pcx_cur[0]
                last_kt = 4 * qc + 3
                goff = 0
                for kt, n, q0 in grp:
                    co = q0 - 512 * qc
                    for a in range(2):
                        nc.tensor.matmul(
                            pcx[a][:, co:co + n],
                            lhsT=Vt[:, kt, 2 * m + a, :],
                            rhs=t['praw'][a][:, goff:goff + n],
                            start=(kt == 0), stop=(kt == last_kt))
                    goff += n
                return pcx

            def emit_norm(qc, m, pcx):
                lr2 = normp.tile([1, 1024], bf16, tag="lr2", bufs=2,
                                 name="lr2")
                for a in range(2):
                    nc.vector.tensor_copy(out=lr2[0:1, 512 * a:512 * a + 512],
                                          in_=pcx[a][DK:DK + 1, :])
                for a in range(2):
                    pb = psum.tile([DK, 512], f32, tag="ps", bufs=3, name="pb")
                    nc.tensor.matmul(pb, lhsT=ones_b,
                                     rhs=lr2[0:1, 512 * a:512 * a + 512],
                                     start=True, stop=True)
                    crec = normp.tile([DK, 512], f32, tag=f"crec{a}", bufs=2,
                                      name=f"crec{a}")
                    nc.vector.reciprocal_approx_fast(out=crec, in_=pb)
                    nc.vector.tensor_mul(
                        out=cxT[m][64 * a:64 * a + DK,
                                   512 * qc:512 * qc + 512],
                        in0=pcx[a][0:DK, :], in1=crec)

            out_q = [nc.sync, nc.gpsimd]

            def emit_outproj(qc):
                for tt in range(4 * qc, 4 * qc + 4):
                    po = psum.tile([128, 1024], f32, tag="ps", bufs=3,
                                   name="po")
                    for m in range(2):
                        for e in range(2):
                            nc.tensor.matmul(
                                po[:, 512 * e:512 * e + 512],
                                lhsT=cxT[m][:, 128 * tt:128 * tt + 128],
                                rhs=wo_s[:, m, 512 * e:512 * e + 512],
                                start=(m == 0), stop=(m == 1))
                    ost = outst.tile([128, D], bf16, tag="ost", name="ost")
                    nc.vector.tensor_copy(out=ost, in_=po)
                    out_q[tt % 2].dma_start(
                        out=dout[128 * tt:128 * tt + 128, :], in_=ost)

            def run_phase(ph_tasks, base_idx, prefetch=()):
                prefetch = list(prefetch)
                prev = None
                for k, t in enumerate(ph_tasks):
                    ensure_eb(base_idx + k)
                    ensure_eb(base_idx + k + 1)
                    ensure_eb(base_idx + k + 2)
                    for _ in range(4):
                        if prefetch:
                            prefetch.pop(0)()
                    emit_scores(t)
                    if prev is not None:
                        pcx = emit_av(prev)
                        if prev['last']:
                            emit_norm(prev['qc'], prev['m'], pcx)
                            if prev['m'] == 1:
                                emit_outproj(prev['qc'])
                    prev = t
                pcx = emit_av(prev)
                if prev['last']:
                    emit_norm(prev['qc'], prev['m'], pcx)
                    if prev['m'] == 1:
                        emit_outproj(prev['qc'])

            # ---- wave 0: projections for q/k columns 0..1023, V tiles 0..7 ----
            for m in range(2):
                qk_proj(m, 0, xq0, wq_s, QT[m])
            for m in range(2):
                qk_proj(m, 0, xk0, wk_s, KTt[m])
            v_proj(0, vw0)

            # prefetch wave-1 inputs, paced against ph0's eb stream
            xq1 = xvp.tile([128, 8, 1024], bf16, tag="xv", bufs=4, name="xq1")
            xk1 = xvp.tile([128, 8, 1024], bf16, tag="xv", bufs=4, name="xk1")
            vw1 = xvp.tile([128, 8, 1024], bf16, tag="xv", bufs=4, name="vw1")
            prefetch = []
            for t in range(8):
                prefetch.append(lambda t=t: nc.sync.dma_start(
                    out=xq1[:, t, :], in_=dqT[t][:, 1024:2048]))
                prefetch.append(lambda t=t: nc.gpsimd.dma_start(
                    out=xk1[:, t, :], in_=dkT[t][:, 1024:2048]))
                prefetch.append(lambda t=t: nc.gpsimd.dma_start(
                    out=vw1[:, t, :], in_=dvT[t][:, 1024:2048]))

            # ---- attention qc0 + qc1 ----
            ph0 = [t for t in tasks if t['qc'] < 2]
            run_phase(ph0, 0, prefetch)
            for fn in prefetch[24:]:
                fn()

            # ---- wave 1: projections for q/k columns 1024..2047, V tiles 8..15
            for m in range(2):
                qk_proj(m, 1, xq1, wq_s, QT[m])
            for m in range(2):
                qk_proj(m, 1, xk1, wk_s, KTt[m])
            v_proj(1, vw1)

            # ---- attention qc2 + qc3 ----
            ph1 = [t for t in tasks if t['qc'] >= 2]
            run_phase(ph1, len(ph0))

    nc.compile()
    return nc


def _get_program():
    global _PROGRAM
    if _PROGRAM is None:
        _PROGRAM = _build_program()
    return _PROGRAM


def _rope_tables():
    half = DK // 2
    inv_freq = 1.0 / (10000.0 ** (np.arange(half, dtype=np.float64) / half))
    ang = np.arange(S, dtype=np.float64)[:, None] * inv_freq[None, :]  # [S, 32]
    cos = np.cos(ang).T  # [32, S]
    sin = np.sin(ang).T
    cos64 = np.concatenate([cos, cos], axis=0)            # [64, S]
    # t1 = x * ssin; dst[0:32] = cc[0:32] + t1[32:64] = x1 cos - x2 sin
    #               dst[32:64] = cc[32:64] + t1[0:32] = x2 cos + x1 sin
    ssin64 = np.concatenate([sin, -sin], axis=0)
    cosT = np.tile(cos64, (2, 1)).astype(BF16)            # [128, S]
    ssinT = np.tile(ssin64, (2, 1)).astype(BF16)
    return np.ascontiguousarray(cosT), np.ascontiguousarray(ssinT)


def _pack_ebias(bias_g):
    """bias_g: [HPC, S, S] f32 (this group's heads). Returns packed 1D bf16,
    one contiguous [128, gn] row-major block per kt-quad (matching the wide
    SBUF tiles the kernel DMAs)."""
    out = np.empty(EB_TOTAL, dtype=BF16)
    off = 0
    tri = np.triu(np.ones((128, 128), dtype=np.float32))
    for h in range(HPC):
        for qc in range(QC):
            for grp in _quads(qc):
                blks = []
                for kt, n, q0 in grp:
                    blk = np.exp(
                        bias_g[h, q0:q0 + n, 128 * kt:128 * kt + 128]
                        .astype(np.float64)).T.astype(np.float32)  # [128, n]
                    if kt // 4 == qc:
                        blk[:, 0:128] *= tri
                    blks.append(blk)
                wide = np.concatenate(blks, axis=1)  # [128, gn]
                gn = wide.shape[1]
                out[off:off + 128 * gn] = wide.astype(BF16).reshape(-1)
                off += 128 * gn
    assert off == EB_TOTAL
    return out


def _prep_inputs(query, key, value, rel_pos_bias, Wq, Wk, Wv, Wo_w):
    cosT, ssinT = _rope_tables()
    xT = {}
    for nm, x in (("q", query), ("k", key), ("v", value)):
        for b in range(B):
            t = np.ascontiguousarray(x[b].T.reshape(8, 128, S)).astype(BF16)
            xT[(nm, b)] = t
    wqs, wks, wvs, wos, ebs = {}, {}, {}, {}, {}
    for g in range(4):
        sl = slice(GDIM * g, GDIM * (g + 1))
        wqs[g] = np.ascontiguousarray(
            (Wq[sl, :] / SCALE).T.reshape(8, 128, GDIM)).astype(BF16)
        wks[g] = np.ascontiguousarray(Wk[sl, :].T.reshape(8, 128, GDIM)).astype(BF16)
        wvs[g] = np.ascontiguousarray(Wv[sl, :].T.reshape(8, 128, GDIM)).astype(BF16)
        wos[g] = np.ascontiguousarray(Wo_w[:, sl].T.reshape(2, 128, D)).astype(BF16)
        ebs[g] = _pack_ebias(rel_pos_bias[0, HPC * g:HPC * (g + 1)])
    in_maps = []
    for c in range(N_CORES):
        b, g = c // 4, c % 4
        in_maps.append({
            "qT": xT[("q", b)], "kT": xT[("k", b)], "vT": xT[("v", b)],
            "wq": wqs[g], "wk": wks[g], "wv": wvs[g], "wo": wos[g],
            "eb": ebs[g], "cosT": cosT, "ssinT": ssinT,
        })
    return in_maps


def _run(query, key, value, rel_pos_bias, Wq, Wk, Wv, Wo_w, Wo_b, trace=False,
         **trace_kwargs):
    nc = _get_program()
    in_maps = _prep_inputs(query, key, value, rel_pos_bias, Wq, Wk, Wv, Wo_w)
    res = run_bass_kernel_spmd(nc, in_maps, core_ids=list(range(N_CORES)),
                               trace=trace, **trace_kwargs)
    out = np.empty((B, S, D), dtype=np.float32)
    for b in range(B):
        acc = res.results[4 * b]["out"].astype(np.float32)
        for g in range(1, 4):
            acc = acc + res.results[4 * b + g]["out"].astype(np.float32)
        out[b] = acc + Wo_b[None, :]
    return out, res


def _cpu_fallback(query, key, value, mask, rel_pos_bias, Wq, Wk, Wv, Wo_w, Wo_b):
    def rope_np(x):
        half = DK // 2
        inv_freq = 1.0 / (10000.0 ** (np.arange(half, dtype=np.float32) / half))
        ang = np.arange(S, dtype=np.float32)[:, None] * inv_freq[None, :]
        cos = np.concatenate([np.cos(ang), np.cos(ang)], axis=-1)[None, None]
        sin = np.concatenate([np.sin(ang), np.sin(ang)], axis=-1)[None, None]
        x1, x2 = x[..., :half], x[..., half:]
        rot = np.concatenate([-x2, x1], axis=-1)
        return x * cos + rot * sin

    q = np.einsum('bsd,ed->bse', query, Wq).reshape(B, S, H, DK).transpose(0, 2, 1, 3)
    k = np.einsum('bsd,ed->bse', key, Wk).reshape(B, S, H, DK).transpose(0, 2, 1, 3)
    v = np.einsum('bsd,ed->bse', value, Wv).reshape(B, S, H, DK).transpose(0, 2, 1, 3)
    q, k = rope_np(q), rope_np(k)
    sc = np.einsum('bhqd,bhkd->bhqk', q, k) / SCALE + rel_pos_bias
    sc = np.where(mask, sc, -np.inf)
    sc = sc - sc.max(axis=-1, keepdims=True)
    e = np.exp(sc)
    attn = e / e.sum(axis=-1, keepdims=True)
    ctx = np.einsum('bhqk,bhkd->bhqd', attn, v)
    ctx = ctx.transpose(0, 2, 1, 3).reshape(B, S, D)
    return (np.einsum('bsd,ed->bse', ctx, Wo_w) + Wo_b).astype(np.float32)


def kernel(query, key, value, mask, rel_pos_bias, Wq, Wk, Wv, Wo_w, Wo_b):
    query = np.asarray(query, dtype=np.float32)
    key = np.asarray(key, dtype=np.float32)
    value = np.asarray(value, dtype=np.float32)
    mask = np.asarray(mask)
    rel_pos_bias = np.asarray(rel_pos_bias, dtype=np.float32)
    Wq = np.asarray(Wq, dtype=np.float32)
    Wk = np.asarray(Wk, dtype=np.float32)
    Wv = np.asarray(Wv, dtype=np.float32)
    Wo_w = np.asarray(Wo_w, dtype=np.float32)
    Wo_b = np.asarray(Wo_b, dtype=np.float32)

    if not np.array_equal(mask.reshape(S, S),
                          np.tril(np.ones((S, S), dtype=bool))):
        return _cpu_fallback(query, key, value, mask, rel_pos_bias,
                             Wq, Wk, Wv, Wo_w, Wo_b)

    out, _ = _run(query, key, value, rel_pos_bias, Wq, Wk, Wv, Wo_w, Wo_b)
    return out



# revision 17
# speedup vs baseline: 1.0467x; 1.0058x over previous
"""Multi-head attention (B=2, S=2048, D=1024, H=16, causal + rel-pos-bias + RoPE)
on 8 Trainium2 NeuronCores.

Sharding: core c handles batch c//4 and head-group c%4 (4 heads = 256 model dims).
Each core computes its heads' Q/K/V projections (column-sharded weights), RoPE,
causal attention with relative position bias, and a partial output projection
(row-sharded Wo). Host sums the 4 partials per batch and adds Wo_b.

v2: wave-split projections interleaved with attention (qc0/qc1 run between the
two projection waves), RoPE without scalar-engine copies (sign-baked sin table
+ partition-block-swap copies on DVE), batched normalization, bf16 output,
deeper PSUM pipelining, prioritized DMA queues.
"""

import math

import numpy as np
import ml_dtypes

import concourse.bass as bass
import concourse.mybir as mybir
import concourse.tile as tile
from concourse import bacc
from concourse.bass_utils import run_bass_kernel_spmd

BF16 = ml_dtypes.bfloat16

B, S, D, H = 2, 2048, 1024, 16
DK = 64
SCALE = math.sqrt(DK)
HPC = 4          # heads per core
GDIM = HPC * DK  # 256 model dims per core
N_CORES = 8
KT = S // 128    # 16 k-tiles
QC = S // 512    # 4 q-chunks

f32 = mybir.dt.float32
f32r = mybir.dt.float32r
bf16 = mybir.dt.bfloat16


def _quads(qc):
    """kt quad-groups for one (h, qc) chunk: list of [(kt,n,q0)...] (len<=4)."""
    kts = list(range(4 * qc + 4))
    out = []
    for i in range(0, len(kts), 4):
        grp = []
        for kt in kts[i:i + 4]:
            if kt // 4 == qc:
                n = 512 - 128 * (kt % 4)
                q0 = 128 * kt
            else:
                n = 512
                q0 = 512 * qc
            grp.append((kt, n, q0))
        out.append(grp)
    return out


def _sched():
    for h in range(HPC):
        for qc in range(QC):
            for grp in _quads(qc):
                for kt, n, q0 in grp:
                    yield h, qc, kt, n, q0


EB_PER_HEAD = sum(128 * n for h, qc, kt, n, q0 in _sched()) // HPC
EB_TOTAL = EB_PER_HEAD * HPC

_PROGRAM = None


def _build_program():
    nc = bacc.Bacc("TRN2", target_bir_lowering=False, debug=False)

    dqT = nc.dram_tensor("qT", [8, 128, S], bf16, kind="ExternalInput").ap()
    dkT = nc.dram_tensor("kT", [8, 128, S], bf16, kind="ExternalInput").ap()
    dvT = nc.dram_tensor("vT", [8, 128, S], bf16, kind="ExternalInput").ap()
    dwq = nc.dram_tensor("wq", [8, 128, GDIM], bf16, kind="ExternalInput").ap()
    dwk = nc.dram_tensor("wk", [8, 128, GDIM], bf16, kind="ExternalInput").ap()
    dwv = nc.dram_tensor("wv", [8, 128, GDIM], bf16, kind="ExternalInput").ap()
    dwo = nc.dram_tensor("wo", [2, 128, D], bf16, kind="ExternalInput").ap()
    deb = nc.dram_tensor("eb", [EB_TOTAL], bf16, kind="ExternalInput").ap()
    dcos = nc.dram_tensor("cosT", [128, S], bf16, kind="ExternalInput").ap()
    dssin = nc.dram_tensor("ssinT", [128, S], bf16, kind="ExternalInput").ap()
    dout = nc.dram_tensor("out", [S, D], bf16, kind="ExternalOutput").ap()

    # per-head packed-bias offset of each qc block
    woff_qc = []
    acc = 0
    for qc in range(QC):
        woff_qc.append(acc)
        for grp in _quads(qc):
            acc += 128 * sum(n for kt, n, q0 in grp)
    assert acc == EB_PER_HEAD

    with tile.TileContext(nc) as tc:
        with tc.tile_pool(name="consts", bufs=1) as consts, \
             tc.tile_pool(name="persist", bufs=1) as persist, \
             tc.tile_pool(name="xvp", bufs=1) as xvp, \
             tc.tile_pool(name="ropep", bufs=1) as ropep, \
             tc.tile_pool(name="attn_sb", bufs=1) as attn_sb, \
             tc.tile_pool(name="normp", bufs=1) as normp, \
             tc.tile_pool(name="outst", bufs=3) as outst, \
             tc.tile_pool(name="psum", bufs=1, space="PSUM") as psum:

            # ---- constants & persistent tiles ----
            warm = consts.tile([128, 512], bf16)
            wq_s = consts.tile([128, 8, GDIM], bf16)
            wk_s = consts.tile([128, 8, GDIM], bf16)
            wv_s = consts.tile([128, 8, GDIM], bf16)
            wo_s = consts.tile([128, 2, D], bf16)
            cos_s = consts.tile([128, S], bf16)
            ssin_s = consts.tile([128, S], bf16)

            QT = [persist.tile([128, S], bf16, name=f"QT{m}") for m in range(2)]
            KTt = [persist.tile([128, S], bf16, name=f"KTt{m}") for m in range(2)]
            Vt = persist.tile([128, KT, HPC, DK + 1], bf16)
            cxT = [persist.tile([128, S], bf16, name=f"cxT{m}") for m in range(2)]
            nc.vector.memset(Vt[:, :, :, DK:DK + 1], 1.0)
            nc.vector.memset(warm, 0.0)

            # ones row for the denominator broadcast matmuls
            ones_b = consts.tile([1, 64], bf16)
            nc.vector.memset(ones_b, 1.0)

            # ---- input DMAs, prioritized ----
            # sync: wq + x-of-Q wave0 (first compute)
            xq0 = xvp.tile([128, 8, 1024], bf16, tag="xv", bufs=4, name="xq0")
            for t in range(8):
                nc.sync.dma_start(out=wq_s[:, t, :], in_=dwq[t])
                nc.sync.dma_start(out=xq0[:, t, :], in_=dqT[t][:, 0:1024])
            # gpsimd: wk + x-of-K wave0
            xk0 = xvp.tile([128, 8, 1024], bf16, tag="xv", bufs=4, name="xk0")
            for t in range(8):
                nc.gpsimd.dma_start(out=wk_s[:, t, :], in_=dwk[t])
                nc.gpsimd.dma_start(out=xk0[:, t, :], in_=dkT[t][:, 0:1024])
            # scalar: rope tables, wv, v wave0
            nc.scalar.dma_start(out=cos_s, in_=dcos)
            nc.scalar.dma_start(out=ssin_s, in_=dssin)
            vw0 = xvp.tile([128, 8, 1024], bf16, tag="xv", bufs=4, name="vw0")
            for t in range(8):
                nc.scalar.dma_start(out=wv_s[:, t, :], in_=dwv[t])
                nc.scalar.dma_start(out=vw0[:, t, :], in_=dvT[t][:, 0:1024])
            # gpsimd: wo (needed from first outproj, ~mid-kernel)
            for t in range(2):
                nc.gpsimd.dma_start(out=wo_s[:, t, :], in_=dwo[t])

            # ---- helpers ----
            def rope(pp_half, dst, n):
                """pp_half: psum [128,512] raw proj.T for 512-col chunk n."""
                t1 = ropep.tile([128, 512], bf16, tag="t1", bufs=3, name="t1")
                nc.vector.tensor_mul(out=t1, in0=pp_half,
                                     in1=ssin_s[:, 512 * n:512 * n + 512])
                # rotate-half: partition-block swap (out-shifted copies are
                # legal on DVE; TT inputs must share a partition base)
                t1s = ropep.tile([128, 512], bf16, tag="t1s", bufs=3,
                                 name="t1s")
                for o, i in ((0, 32), (32, 0), (64, 96), (96, 64)):
                    nc.vector.tensor_copy(out=t1s[o:o + 32, :],
                                          in_=t1[i:i + 32, :])
                ds = dst[:, 512 * n:512 * n + 512]
                nc.vector.tensor_mul(out=ds, in0=pp_half,
                                     in1=cos_s[:, 512 * n:512 * n + 512])
                nc.vector.tensor_add(out=ds, in0=ds, in1=t1s)

            def qk_proj(m, w, xw, wsrc, dst):
                pp = psum.tile([128, 1024], f32, tag="ps", bufs=3, name="pp")
                for t in range(8):
                    for j in range(2):
                        nc.tensor.matmul(
                            pp[:, 512 * j:512 * j + 512],
                            lhsT=wsrc[:, t, 128 * m:128 * m + 128],
                            rhs=xw[:, t, 512 * j:512 * j + 512],
                            start=(t == 0), stop=(t == 7))
                for j in range(2):
                    rope(pp[:, 512 * j:512 * j + 512], dst, 2 * w + j)

            def v_proj(w, vww):
                for tt in range(8 * w, 8 * w + 8):
                    pv = psum.tile([128, GDIM], f32, tag="ps", bufs=3,
                                   name="pv")
                    for t in range(8):
                        nc.tensor.matmul(
                            pv,
                            lhsT=vww[:, t,
                                     128 * (tt - 8 * w):128 * (tt - 8 * w) + 128],
                            rhs=wv_s[:, t, :],
                            start=(t == 0), stop=(t == 7))
                    nc.scalar.copy(
                        out=Vt[:, tt, :, 0:DK],
                        in_=pv.rearrange("p (h d) -> p h d", h=HPC))

            # ---- attention task list ----
            tasks = []
            for qc in range(QC):
                for m in range(2):
                    qs = _quads(qc)
                    woff = woff_qc[qc]
                    for qi, grp in enumerate(qs):
                        tasks.append(dict(
                            qc=qc, m=m, grp=grp, woff=woff,
                            first=(qi == 0), last=(qi == len(qs) - 1)))
                        woff += 128 * sum(n for kt, n, q0 in grp)

            eb_emitted = [False] * len(tasks)
            eb_q = [nc.sync, nc.gpsimd]

            def ensure_eb(i):
                if i >= len(tasks) or eb_emitted[i]:
                    return
                t = tasks[i]
                gn = sum(n for kt, n, q0 in t['grp'])
                ebt = []
                for a in range(2):
                    e = attn_sb.tile([128, gn], bf16, tag=f"ebt{a}", bufs=3,
                                     name=f"ebt{a}")
                    base = (2 * t['m'] + a) * EB_PER_HEAD + t['woff']
                    eb_q[(i + a) % 2].dma_start(
                        out=e,
                        in_=deb[base:base + 128 * gn].rearrange(
                            "(p n) -> p n", p=128))
                    ebt.append(e)
                t['ebt'] = ebt
                eb_emitted[i] = True

            pcx_cur = [None]

            def emit_scores(t):
                m, grp = t['m'], t['grp']
                gn = sum(n for kt, n, q0 in grp)
                praw = [attn_sb.tile([128, gn], bf16, tag=f"praw{a}", bufs=3,
                                     name=f"praw{a}") for a in range(2)]
                goff = 0
                for pi in range(0, len(grp), 2):
                    pair = grp[pi:pi + 2]
                    pn = sum(n for kt, n, q0 in pair)
                    pss = [psum.tile([128, pn], f32, tag="ps", bufs=3,
                                     name=f"pss{a}") for a in range(2)]
                    for a in range(2):
                        soff = 0
                        for kt, n, q0 in pair:
                            nc.tensor.matmul(
                                pss[a][:, soff:soff + n],
                                lhsT=KTt[m][64 * a:64 * a + DK,
                                            128 * kt:128 * kt + 128],
                                rhs=QT[m][64 * a:64 * a + DK, q0:q0 + n],
                                start=True, stop=True,
                                tile_position=(64 * a, 0))
                            soff += n
                    for a in range(2):
                        nc.scalar.activation(
                            out=praw[a][:, goff:goff + pn], in_=pss[a],
                            func=mybir.ActivationFunctionType.Exp)
                    goff += pn
                for a in range(2):
                    nc.vector.tensor_mul(out=praw[a], in0=praw[a],
                                         in1=t['ebt'][a])
                t['praw'] = praw

            def emit_av(t):
                qc, m, grp = t['qc'], t['m'], t['grp']
                if t['first']:
                    pcx_cur[0] = [psum.tile([DK + 1, 512], f32, tag="pcx",
                                            bufs=2, name=f"pcx{a}")
                                  for a in range(2)]
                pcx = pcx_cur[0]
                last_kt = 4 * qc + 3
                goff = 0
                for kt, n, q0 in grp:
                    co = q0 - 512 * qc
                    for a in range(2):
                        nc.tensor.matmul(
                            pcx[a][:, co:co + n],
                            lhsT=Vt[:, kt, 2 * m + a, :],
                            rhs=t['praw'][a][:, goff:goff + n],
                            start=(kt == 0), stop=(kt == last_kt))
                    goff += n
                return pcx

            def emit_norm(qc, m, pcx):
                lr2 = normp.tile([1, 1024], bf16, tag="lr2", bufs=2,
                                 name="lr2")
                for a in range(2):
                    nc.vector.tensor_copy(out=lr2[0:1, 512 * a:512 * a + 512],
                                          in_=pcx[a][DK:DK + 1, :])
                for a in range(2):
                    pb = psum.tile([DK, 512], f32, tag="ps", bufs=3, name="pb")
                    nc.tensor.matmul(pb, lhsT=ones_b,
                                     rhs=lr2[0:1, 512 * a:512 * a + 512],
                                     start=True, stop=True)
                    crec = normp.tile([DK, 512], f32, tag=f"crec{a}", bufs=2,
                                      name=f"crec{a}")
                    nc.vector.reciprocal_approx_fast(out=crec, in_=pb)
                    nc.vector.tensor_mul(
                        out=cxT[m][64 * a:64 * a + DK,
                                   512 * qc:512 * qc + 512],
                        in0=pcx[a][0:DK, :], in1=crec)

            out_q = [nc.sync, nc.gpsimd]

            def emit_outproj(qc):
                for tt in range(4 * qc, 4 * qc + 4):
                    po = psum.tile([128, 1024], f32, tag="ps", bufs=3,
                                   name="po")
                    for m in range(2):
                        for e in range(2):
                            nc.tensor.matmul(
                                po[:, 512 * e:512 * e + 512],
                                lhsT=cxT[m][:, 128 * tt:128 * tt + 128],
                                rhs=wo_s[:, m, 512 * e:512 * e + 512],
                                start=(m == 0), stop=(m == 1))
                    ost = outst.tile([128, D], bf16, tag="ost", name="ost")
                    nc.vector.tensor_copy(out=ost, in_=po)
                    out_q[tt % 2].dma_start(
                        out=dout[128 * tt:128 * tt + 128, :], in_=ost)

            def run_phase(ph_tasks, base_idx, prefetch=()):
                prefetch = list(prefetch)
                prev = None
                for k, t in enumerate(ph_tasks):
                    ensure_eb(base_idx + k)
                    ensure_eb(base_idx + k + 1)
                    ensure_eb(base_idx + k + 2)
                    for _ in range(4):
                        if prefetch:
                            prefetch.pop(0)()
                    emit_scores(t)
                    if prev is not None:
                        pcx = emit_av(prev)
                        if prev['last']:
                            emit_norm(prev['qc'], prev['m'], pcx)
                            if prev['m'] == 1:
                                emit_outproj(prev['qc'])
                    prev = t
                pcx = emit_av(prev)
                if prev['last']:
                    emit_norm(prev['qc'], prev['m'], pcx)
                    if prev['m'] == 1:
                        emit_outproj(prev['qc'])

            # ---- wave 0: projections for q/k columns 0..1023, V tiles 0..7 ----
            for m in range(2):
                qk_proj(m, 0, xq0, wq_s, QT[m])
            for m in range(2):
                qk_proj(m, 0, xk0, wk_s, KTt[m])
            v_proj(0, vw0)

            # prefetch wave-1 inputs, paced against ph0's eb stream
            xq1 = xvp.tile([128, 8, 1024], bf16, tag="xv", bufs=4, name="xq1")
            xk1 = xvp.tile([128, 8, 1024], bf16, tag="xv", bufs=4, name="xk1")
            vw1 = xvp.tile([128, 8, 1024], bf16, tag="xv", bufs=4, name="vw1")
            prefetch = []
            for t in range(8):
                prefetch.append(lambda t=t: nc.sync.dma_start(
                    out=xq1[:, t, :], in_=dqT[t][:, 1024:2048]))
                prefetch.append(lambda t=t: nc.gpsimd.dma_start(
                    out=xk1[:, t, :], in_=dkT[t][:, 1024:2048]))
                prefetch.append(lambda t=t: nc.gpsimd.dma_start(
                    out=vw1[:, t, :], in_=dvT[t][:, 1024:2048]))

            # ---- attention qc0 + qc1 ----
            ph0 = [t for t in tasks if t['qc'] < 2]
            run_phase(ph0, 0, prefetch)
            for fn in prefetch[24:]:
                fn()

            # ---- wave 1: projections for q/k columns 1024..2047, V tiles 8..15
            for m in range(2):
                qk_proj(m, 1, xq1, wq_s, QT[m])
            for m in range(2):
                qk_proj(m, 1, xk1, wk_s, KTt[m])
            v_proj(1, vw1)

            # ---- attention qc2 + qc3 ----
            ph1 = [t for t in tasks if t['qc'] >= 2]
            run_phase(ph1, len(ph0))

    nc.compile()
    return nc


def _get_program():
    global _PROGRAM
    if _PROGRAM is None:
        _PROGRAM = _build_program()
    return _PROGRAM


def _rope_tables():
    half = DK // 2
    inv_freq = 1.0 / (10000.0 ** (np.arange(half, dtype=np.float64) / half))
    ang = np.arange(S, dtype=np.float64)[:, None] * inv_freq[None, :]  # [S, 32]
    cos = np.cos(ang).T  # [32, S]
    sin = np.sin(ang).T
    cos64 = np.concatenate([cos, cos], axis=0)            # [64, S]
    # t1 = x * ssin; dst[0:32] = cc[0:32] + t1[32:64] = x1 cos - x2 sin
    #               dst[32:64] = cc[32:64] + t1[0:32] = x2 cos + x1 sin
    ssin64 = np.concatenate([sin, -sin], axis=0)
    cosT = np.tile(cos64, (2, 1)).astype(BF16)            # [128, S]
    ssinT = np.tile(ssin64, (2, 1)).astype(BF16)
    return np.ascontiguousarray(cosT), np.ascontiguousarray(ssinT)


def _pack_ebias(bias_g):
    """bias_g: [HPC, S, S] f32 (this group's heads). Returns packed 1D bf16,
    one contiguous [128, gn] row-major block per kt-quad (matching the wide
    SBUF tiles the kernel DMAs)."""
    out = np.empty(EB_TOTAL, dtype=BF16)
    off = 0
    tri = np.triu(np.ones((128, 128), dtype=np.float32))
    for h in range(HPC):
        for qc in range(QC):
            for grp in _quads(qc):
                blks = []
                for kt, n, q0 in grp:
                    blk = np.exp(
                        bias_g[h, q0:q0 + n, 128 * kt:128 * kt + 128]
                        .astype(np.float64)).T.astype(np.float32)  # [128, n]
                    if kt // 4 == qc:
                        blk[:, 0:128] *= tri
                    blks.append(blk)
                wide = np.concatenate(blks, axis=1)  # [128, gn]
                gn = wide.shape[1]
                out[off:off + 128 * gn] = wide.astype(BF16).reshape(-1)
                off += 128 * gn
    assert off == EB_TOTAL
    return out


def _prep_inputs(query, key, value, rel_pos_bias, Wq, Wk, Wv, Wo_w):
    cosT, ssinT = _rope_tables()
    xT = {}
    for nm, x in (("q", query), ("k", key), ("v", value)):
        for b in range(B):
            t = np.ascontiguousarray(x[b].T.reshape(8, 128, S)).astype(BF16)
            xT[(nm, b)] = t
    wqs, wks, wvs, wos, ebs = {}, {}, {}, {}, {}
    for g in range(4):
        sl = slice(GDIM * g, GDIM * (g + 1))
        wqs[g] = np.ascontiguousarray(
            (Wq[sl, :] / SCALE).T.reshape(8, 128, GDIM)).astype(BF16)
        wks[g] = np.ascontiguousarray(Wk[sl, :].T.reshape(8, 128, GDIM)).astype(BF16)
        wvs[g] = np.ascontiguousarray(Wv[sl, :].T.reshape(8, 128, GDIM)).astype(BF16)
        wos[g] = np.ascontiguousarray(Wo_w[:, sl].T.reshape(2, 128, D)).astype(BF16)
        ebs[g] = _pack_ebias(rel_pos_bias[0, HPC * g:HPC * (g + 1)])
    in_maps = []
    for c in range(N_CORES):
        b, g = c // 4, c % 4
        in_maps.append({
            "qT": xT[("q", b)], "kT": xT[("k", b)], "vT": xT[("v", b)],
            "wq": wqs[g], "wk": wks[g], "wv": wvs[g], "wo": wos[g],
            "eb": ebs[g], "cosT": cosT, "ssinT": ssinT,
        })
    return in_maps


def _run(query, key, value, rel_pos_bias, Wq, Wk, Wv, Wo_w, Wo_b, trace=False,
         **trace_kwargs):
    nc = _get_program()
    in_maps = _prep_inputs(query, key, value, rel_pos_bias, Wq, Wk, Wv, Wo_w)
    res = run_bass_kernel_spmd(nc, in_maps, core_ids=list(range(N_CORES)),
                               trace=trace, **trace_kwargs)
    out = np.empty((B, S, D), dtype=np.float32)
    for b in range(B):
        acc = res.results[4 * b]["out"].astype(np.float32)
        for g in range(1, 4):
            acc = acc + res.results[4 * b + g]["out"].astype(np.float32)
        out[b] = acc + Wo_b[None, :]
    return out, res


def _cpu_fallback(query, key, value, mask, rel_pos_bias, Wq, Wk, Wv, Wo_w, Wo_b):
    def rope_np(x):
        half = DK // 2
        inv_freq = 1.0 / (10000.0 ** (np.arange(half, dtype=np.float32) / half))
        ang = np.arange(S, dtype=np.float32)[:, None] * inv_freq[None, :]
        cos = np.concatenate([np.cos(ang), np.cos(ang)], axis=-1)[None, None]
        sin = np.concatenate([np.sin(ang), np.sin(ang)], axis=-1)[None, None]
        x1, x2 = x[..., :half], x[..., half:]
        rot = np.concatenate([-x2, x1], axis=-1)
        return x * cos + rot * sin

    q = np.einsum('bsd,ed->bse', query, Wq).reshape(B, S, H, DK).transpose(0, 2, 1, 3)
    k = np.einsum('bsd,ed->bse', key, Wk).reshape(B, S, H, DK).transpose(0, 2, 1, 3)
    v = np.einsum('bsd,ed->bse', value, Wv).reshape(B, S, H, DK).transpose(0, 2, 1, 3)
    q, k = rope_np(q), rope_np(k)
    sc = np.einsum('bhqd,bhkd->bhqk', q, k) / SCALE + rel_pos_bias
    sc = np.where(mask, sc, -np.inf)
    sc = sc - sc.max(axis=-1, keepdims=True)
    e = np.exp(sc)
    attn = e / e.sum(axis=-1, keepdims=True)
    ctx = np.einsum('bhqk,bhkd->bhqd', attn, v)
    ctx = ctx.transpose(0, 2, 1, 3).reshape(B, S, D)
    return (np.einsum('bsd,ed->bse', ctx, Wo_w) + Wo_b).astype(np.float32)


def kernel(query, key, value, mask, rel_pos_bias, Wq, Wk, Wv, Wo_w, Wo_b):
    query = np.asarray(query, dtype=np.float32)
    key = np.asarray(key, dtype=np.float32)
    value = np.asarray(value, dtype=np.float32)
    mask = np.asarray(mask)
    rel_pos_bias = np.asarray(rel_pos_bias, dtype=np.float32)
    Wq = np.asarray(Wq, dtype=np.float32)
    Wk = np.asarray(Wk, dtype=np.float32)
    Wv = np.asarray(Wv, dtype=np.float32)
    Wo_w = np.asarray(Wo_w, dtype=np.float32)
    Wo_b = np.asarray(Wo_b, dtype=np.float32)

    if not np.array_equal(mask.reshape(S, S),
                          np.tril(np.ones((S, S), dtype=bool))):
        return _cpu_fallback(query, key, value, mask, rel_pos_bias,
                             Wq, Wk, Wv, Wo_w, Wo_b)

    out, _ = _run(query, key, value, rel_pos_bias, Wq, Wk, Wv, Wo_w, Wo_b)
    return out



# revision 18
# speedup vs baseline: 1.0589x; 1.0117x over previous
"""Multi-head attention (B=2, S=2048, D=1024, H=16, causal + rel-pos-bias + RoPE)
on 8 Trainium2 NeuronCores.

Sharding: core c handles batch c//4 and head-group c%4 (4 heads = 256 model dims).
Each core computes its heads' Q/K/V projections (column-sharded weights), RoPE,
causal attention with relative position bias, and a partial output projection
(row-sharded Wo). Host sums the 4 partials per batch and adds Wo_b.

v2: wave-split projections interleaved with attention (qc0/qc1 run between the
two projection waves), RoPE without scalar-engine copies (sign-baked sin table
+ partition-block-swap copies on DVE), batched normalization, bf16 output,
deeper PSUM pipelining, prioritized DMA queues.
"""

import math

import numpy as np
import ml_dtypes

import concourse.bass as bass
import concourse.mybir as mybir
import concourse.tile as tile
from concourse import bacc
from concourse.bass_utils import run_bass_kernel_spmd

BF16 = ml_dtypes.bfloat16

B, S, D, H = 2, 2048, 1024, 16
DK = 64
SCALE = math.sqrt(DK)
HPC = 4          # heads per core
GDIM = HPC * DK  # 256 model dims per core
N_CORES = 8
KT = S // 128    # 16 k-tiles
QC = S // 512    # 4 q-chunks

f32 = mybir.dt.float32
f32r = mybir.dt.float32r
bf16 = mybir.dt.bfloat16


def _quads(qc):
    """kt quad-groups for one (h, qc) chunk: list of [(kt,n,q0)...] (len<=4)."""
    kts = list(range(4 * qc + 4))
    out = []
    for i in range(0, len(kts), 4):
        grp = []
        for kt in kts[i:i + 4]:
            if kt // 4 == qc:
                n = 512 - 128 * (kt % 4)
                q0 = 128 * kt
            else:
                n = 512
                q0 = 512 * qc
            grp.append((kt, n, q0))
        out.append(grp)
    return out


def _sched():
    for h in range(HPC):
        for qc in range(QC):
            for grp in _quads(qc):
                for kt, n, q0 in grp:
                    yield h, qc, kt, n, q0


EB_PER_HEAD = sum(128 * n for h, qc, kt, n, q0 in _sched()) // HPC
EB_TOTAL = EB_PER_HEAD * HPC

_PROGRAM = None


def _build_program():
    nc = bacc.Bacc("TRN2", target_bir_lowering=False, debug=False)

    dqT = nc.dram_tensor("qT", [8, 128, S], bf16, kind="ExternalInput").ap()
    dkT = nc.dram_tensor("kT", [8, 128, S], bf16, kind="ExternalInput").ap()
    dvT = nc.dram_tensor("vT", [8, 128, S], bf16, kind="ExternalInput").ap()
    dwq = nc.dram_tensor("wq", [8, 128, GDIM], bf16, kind="ExternalInput").ap()
    dwk = nc.dram_tensor("wk", [8, 128, GDIM], bf16, kind="ExternalInput").ap()
    dwv = nc.dram_tensor("wv", [8, 128, GDIM], bf16, kind="ExternalInput").ap()
    dwo = nc.dram_tensor("wo", [2, 128, D], bf16, kind="ExternalInput").ap()
    deb = nc.dram_tensor("eb", [EB_TOTAL], bf16, kind="ExternalInput").ap()
    dcos = nc.dram_tensor("cosT", [128, S], bf16, kind="ExternalInput").ap()
    dssin = nc.dram_tensor("ssinT", [128, S], bf16, kind="ExternalInput").ap()
    dout = nc.dram_tensor("out", [S, D], bf16, kind="ExternalOutput").ap()

    # per-head packed-bias offset of each qc block
    woff_qc = []
    acc = 0
    for qc in range(QC):
        woff_qc.append(acc)
        for grp in _quads(qc):
            acc += 128 * sum(n for kt, n, q0 in grp)
    assert acc == EB_PER_HEAD

    with tile.TileContext(nc) as tc:
        with tc.tile_pool(name="consts", bufs=1) as consts, \
             tc.tile_pool(name="persist", bufs=1) as persist, \
             tc.tile_pool(name="xvp", bufs=1) as xvp, \
             tc.tile_pool(name="ropep", bufs=1) as ropep, \
             tc.tile_pool(name="attn_sb", bufs=1) as attn_sb, \
             tc.tile_pool(name="normp", bufs=1) as normp, \
             tc.tile_pool(name="outst", bufs=3) as outst, \
             tc.tile_pool(name="psum", bufs=1, space="PSUM") as psum:

            # ---- constants & persistent tiles ----
            wq_s = consts.tile([128, 8, GDIM], bf16)
            wk_s = consts.tile([128, 8, GDIM], bf16)
            wv_s = consts.tile([128, 8, GDIM], bf16)
            wo_s = consts.tile([128, 2, D], bf16)
            cos_s = consts.tile([128, S], bf16)
            ssin_s = consts.tile([128, S], bf16)

            QT = [persist.tile([128, S], bf16, name=f"QT{m}") for m in range(2)]
            KTt = [persist.tile([128, S], bf16, name=f"KTt{m}") for m in range(2)]
            Vt = persist.tile([128, KT, HPC, DK + 1], bf16)
            cxT = [persist.tile([128, S], bf16, name=f"cxT{m}") for m in range(2)]
            nc.vector.memset(Vt[:, :, :, DK:DK + 1], 1.0)

            # ones row for the denominator broadcast matmuls
            ones_b = consts.tile([1, 64], bf16)
            nc.vector.memset(ones_b, 1.0)

            # ---- input DMAs, prioritized ----
            # sync: wq + x-of-Q wave0 (first compute)
            xq0 = xvp.tile([128, 8, 1024], bf16, tag="xv", bufs=4, name="xq0")
            for t in range(8):
                nc.sync.dma_start(out=wq_s[:, t, :], in_=dwq[t])
                nc.sync.dma_start(out=xq0[:, t, :], in_=dqT[t][:, 0:1024])
            # gpsimd: wk + x-of-K wave0
            xk0 = xvp.tile([128, 8, 1024], bf16, tag="xv", bufs=4, name="xk0")
            for t in range(8):
                nc.gpsimd.dma_start(out=wk_s[:, t, :], in_=dwk[t])
                nc.gpsimd.dma_start(out=xk0[:, t, :], in_=dkT[t][:, 0:1024])
            # scalar: rope tables, wv, v wave0
            nc.scalar.dma_start(out=cos_s, in_=dcos)
            nc.scalar.dma_start(out=ssin_s, in_=dssin)
            vw0 = xvp.tile([128, 8, 1024], bf16, tag="xv", bufs=4, name="vw0")
            for t in range(8):
                nc.scalar.dma_start(out=wv_s[:, t, :], in_=dwv[t])
                nc.scalar.dma_start(out=vw0[:, t, :], in_=dvT[t][:, 0:1024])
            # gpsimd: wo (needed from first outproj, ~mid-kernel)
            for t in range(2):
                nc.gpsimd.dma_start(out=wo_s[:, t, :], in_=dwo[t])

            # ---- helpers ----
            def rope(pp_half, dst, n):
                """pp_half: psum [128,512] raw proj.T for 512-col chunk n."""
                t1 = ropep.tile([128, 512], bf16, tag="t1", bufs=3, name="t1")
                nc.vector.tensor_mul(out=t1, in0=pp_half,
                                     in1=ssin_s[:, 512 * n:512 * n + 512])
                # rotate-half: partition-block swap (out-shifted copies are
                # legal on DVE; TT inputs must share a partition base)
                t1s = ropep.tile([128, 512], bf16, tag="t1s", bufs=3,
                                 name="t1s")
                for o, i in ((0, 32), (32, 0), (64, 96), (96, 64)):
                    nc.vector.tensor_copy(out=t1s[o:o + 32, :],
                                          in_=t1[i:i + 32, :])
                ds = dst[:, 512 * n:512 * n + 512]
                nc.vector.tensor_mul(out=ds, in0=pp_half,
                                     in1=cos_s[:, 512 * n:512 * n + 512])
                nc.vector.tensor_add(out=ds, in0=ds, in1=t1s)

            def qk_proj(m, w, xw, wsrc, dst):
                pp = psum.tile([128, 1024], f32, tag="ps", bufs=3, name="pp")
                for t in range(8):
                    for j in range(2):
                        nc.tensor.matmul(
                            pp[:, 512 * j:512 * j + 512],
                            lhsT=wsrc[:, t, 128 * m:128 * m + 128],
                            rhs=xw[:, t, 512 * j:512 * j + 512],
                            start=(t == 0), stop=(t == 7))
                for j in range(2):
                    rope(pp[:, 512 * j:512 * j + 512], dst, 2 * w + j)

            def v_proj(w, vww):
                for tt in range(8 * w, 8 * w + 8):
                    pv = psum.tile([128, GDIM], f32, tag="ps", bufs=3,
                                   name="pv")
                    for t in range(8):
                        nc.tensor.matmul(
                            pv,
                            lhsT=vww[:, t,
                                     128 * (tt - 8 * w):128 * (tt - 8 * w) + 128],
                            rhs=wv_s[:, t, :],
                            start=(t == 0), stop=(t == 7))
                    nc.scalar.copy(
                        out=Vt[:, tt, :, 0:DK],
                        in_=pv.rearrange("p (h d) -> p h d", h=HPC))

            # ---- attention task list ----
            tasks = []
            for qc in range(QC):
                for m in range(2):
                    qs = _quads(qc)
                    woff = woff_qc[qc]
                    for qi, grp in enumerate(qs):
                        tasks.append(dict(
                            qc=qc, m=m, grp=grp, woff=woff,
                            first=(qi == 0), last=(qi == len(qs) - 1)))
                        woff += 128 * sum(n for kt, n, q0 in grp)

            eb_emitted = [False] * len(tasks)
            eb_q = [nc.sync, nc.scalar]

            def ensure_eb(i):
                if i >= len(tasks) or eb_emitted[i]:
                    return
                t = tasks[i]
                gn = sum(n for kt, n, q0 in t['grp'])
                ebt = []
                for a in range(2):
                    e = attn_sb.tile([128, gn], bf16, tag=f"ebt{a}", bufs=3,
                                     name=f"ebt{a}")
                    base = (2 * t['m'] + a) * EB_PER_HEAD + t['woff']
                    eb_q[(i + a) % 2].dma_start(
                        out=e,
                        in_=deb[base:base + 128 * gn].rearrange(
                            "(p n) -> p n", p=128))
                    ebt.append(e)
                t['ebt'] = ebt
                eb_emitted[i] = True

            pcx_cur = [None]

            def emit_scores(t):
                m, grp = t['m'], t['grp']
                gn = sum(n for kt, n, q0 in grp)
                praw = [attn_sb.tile([128, gn], bf16, tag=f"praw{a}", bufs=3,
                                     name=f"praw{a}") for a in range(2)]
                goff = 0
                for pi in range(0, len(grp), 2):
                    pair = grp[pi:pi + 2]
                    pn = sum(n for kt, n, q0 in pair)
                    pss = [psum.tile([128, pn], f32, tag="ps", bufs=3,
                                     name=f"pss{a}") for a in range(2)]
                    for a in range(2):
                        soff = 0
                        for kt, n, q0 in pair:
                            nc.tensor.matmul(
                                pss[a][:, soff:soff + n],
                                lhsT=KTt[m][64 * a:64 * a + DK,
                                            128 * kt:128 * kt + 128],
                                rhs=QT[m][64 * a:64 * a + DK, q0:q0 + n],
                                start=True, stop=True,
                                tile_position=(64 * a, 0))
                            soff += n
                    for a in range(2):
                        nc.scalar.activation(
                            out=praw[a][:, goff:goff + pn], in_=pss[a],
                            func=mybir.ActivationFunctionType.Exp)
                    goff += pn
                for a in range(2):
                    nc.vector.tensor_mul(out=praw[a], in0=praw[a],
                                         in1=t['ebt'][a])
                t['praw'] = praw

            def emit_av(t):
                qc, m, grp = t['qc'], t['m'], t['grp']
                if t['first']:
                    pcx_cur[0] = [psum.tile([DK + 1, 512], f32, tag="pcx",
                                            bufs=2, name=f"pcx{a}")
                                  for a in range(2)]
                pcx = pcx_cur[0]
                last_kt = 4 * qc + 3
                goff = 0
                for kt, n, q0 in grp:
                    co = q0 - 512 * qc
                    for a in range(2):
                        nc.tensor.matmul(
                            pcx[a][:, co:co + n],
                            lhsT=Vt[:, kt, 2 * m + a, :],
                            rhs=t['praw'][a][:, goff:goff + n],
                            start=(kt == 0), stop=(kt == last_kt))
                    goff += n
                return pcx

            def emit_norm(qc, m, pcx):
                lr2 = normp.tile([1, 1024], bf16, tag="lr2", bufs=2,
                                 name="lr2")
                for a in range(2):
                    nc.vector.tensor_copy(out=lr2[0:1, 512 * a:512 * a + 512],
                                          in_=pcx[a][DK:DK + 1, :])
                for a in range(2):
                    pb = psum.tile([DK, 512], f32, tag="ps", bufs=3, name="pb")
                    nc.tensor.matmul(pb, lhsT=ones_b,
                                     rhs=lr2[0:1, 512 * a:512 * a + 512],
                                     start=True, stop=True)
                    crec = normp.tile([DK, 512], f32, tag=f"crec{a}", bufs=2,
                                      name=f"crec{a}")
                    nc.vector.reciprocal_approx_fast(out=crec, in_=pb)
                    nc.vector.tensor_mul(
                        out=cxT[m][64 * a:64 * a + DK,
                                   512 * qc:512 * qc + 512],
                        in0=pcx[a][0:DK, :], in1=crec)

            out_q = [nc.sync, nc.scalar]

            def emit_outproj(qc):
                for tt in range(4 * qc, 4 * qc + 4):
                    po = psum.tile([128, 1024], f32, tag="ps", bufs=3,
                                   name="po")
                    for m in range(2):
                        for e in range(2):
                            nc.tensor.matmul(
                                po[:, 512 * e:512 * e + 512],
                                lhsT=cxT[m][:, 128 * tt:128 * tt + 128],
                                rhs=wo_s[:, m, 512 * e:512 * e + 512],
                                start=(m == 0), stop=(m == 1))
                    ost = outst.tile([128, D], bf16, tag="ost", name="ost")
                    nc.vector.tensor_copy(out=ost, in_=po)
                    out_q[tt % 2].dma_start(
                        out=dout[128 * tt:128 * tt + 128, :], in_=ost)

            def run_phase(ph_tasks, base_idx):
                prev = None
                for k, t in enumerate(ph_tasks):
                    ensure_eb(base_idx + k)
                    ensure_eb(base_idx + k + 1)
                    ensure_eb(base_idx + k + 2)
                    emit_scores(t)
                    if prev is not None:
                        pcx = emit_av(prev)
                        if prev['last']:
                            emit_norm(prev['qc'], prev['m'], pcx)
                            if prev['m'] == 1:
                                emit_outproj(prev['qc'])
                    prev = t
                pcx = emit_av(prev)
                if prev['last']:
                    emit_norm(prev['qc'], prev['m'], pcx)
                    if prev['m'] == 1:
                        emit_outproj(prev['qc'])

            # ---- wave 0: projections for q/k columns 0..1023, V tiles 0..7 ----
            for m in range(2):
                qk_proj(m, 0, xq0, wq_s, QT[m])
            for m in range(2):
                qk_proj(m, 0, xk0, wk_s, KTt[m])
            v_proj(0, vw0)

            # prefetch wave-1 inputs (slots reuse wave-0 x tiles; deps tracked)
            xq1 = xvp.tile([128, 8, 1024], bf16, tag="xv", bufs=4, name="xq1")
            for t in range(8):
                nc.sync.dma_start(out=xq1[:, t, :], in_=dqT[t][:, 1024:2048])
            xk1 = xvp.tile([128, 8, 1024], bf16, tag="xv", bufs=4, name="xk1")
            for t in range(8):
                nc.gpsimd.dma_start(out=xk1[:, t, :], in_=dkT[t][:, 1024:2048])
            vw1 = xvp.tile([128, 8, 1024], bf16, tag="xv", bufs=4, name="vw1")
            for t in range(8):
                nc.gpsimd.dma_start(out=vw1[:, t, :], in_=dvT[t][:, 1024:2048])

            # ---- attention qc0 + qc1 ----
            ph0 = [t for t in tasks if t['qc'] < 2]
            run_phase(ph0, 0)

            # ---- wave 1: projections for q/k columns 1024..2047, V tiles 8..15
            for m in range(2):
                qk_proj(m, 1, xq1, wq_s, QT[m])
            for m in range(2):
                qk_proj(m, 1, xk1, wk_s, KTt[m])
            v_proj(1, vw1)

            # ---- attention qc2 + qc3 ----
            ph1 = [t for t in tasks if t['qc'] >= 2]
            run_phase(ph1, len(ph0))

    nc.compile()
    return nc


def _get_program():
    global _PROGRAM
    if _PROGRAM is None:
        _PROGRAM = _build_program()
    return _PROGRAM


def _rope_tables():
    half = DK // 2
    inv_freq = 1.0 / (10000.0 ** (np.arange(half, dtype=np.float64) / half))
    ang = np.arange(S, dtype=np.float64)[:, None] * inv_freq[None, :]  # [S, 32]
    cos = np.cos(ang).T  # [32, S]
    sin = np.sin(ang).T
    cos64 = np.concatenate([cos, cos], axis=0)            # [64, S]
    # t1 = x * ssin; dst[0:32] = cc[0:32] + t1[32:64] = x1 cos - x2 sin
    #               dst[32:64] = cc[32:64] + t1[0:32] = x2 cos + x1 sin
    ssin64 = np.concatenate([sin, -sin], axis=0)
    cosT = np.tile(cos64, (2, 1)).astype(BF16)            # [128, S]
    ssinT = np.tile(ssin64, (2, 1)).astype(BF16)
    return np.ascontiguousarray(cosT), np.ascontiguousarray(ssinT)


def _pack_ebias(bias_g):
    """bias_g: [HPC, S, S] f32 (this group's heads). Returns packed 1D bf16,
    one contiguous [128, gn] row-major block per kt-quad (matching the wide
    SBUF tiles the kernel DMAs)."""
    out = np.empty(EB_TOTAL, dtype=BF16)
    off = 0
    tri = np.triu(np.ones((128, 128), dtype=np.float32))
    for h in range(HPC):
        for qc in range(QC):
            for grp in _quads(qc):
                blks = []
                for kt, n, q0 in grp:
                    blk = np.exp(
                        bias_g[h, q0:q0 + n, 128 * kt:128 * kt + 128]
                        .astype(np.float64)).T.astype(np.float32)  # [128, n]
                    if kt // 4 == qc:
                        blk[:, 0:128] *= tri
                    blks.append(blk)
                wide = np.concatenate(blks, axis=1)  # [128, gn]
                gn = wide.shape[1]
                out[off:off + 128 * gn] = wide.astype(BF16).reshape(-1)
                off += 128 * gn
    assert off == EB_TOTAL
    return out


def _prep_inputs(query, key, value, rel_pos_bias, Wq, Wk, Wv, Wo_w):
    cosT, ssinT = _rope_tables()
    xT = {}
    for nm, x in (("q", query), ("k", key), ("v", value)):
        for b in range(B):
            t = np.ascontiguousarray(x[b].T.reshape(8, 128, S)).astype(BF16)
            xT[(nm, b)] = t
    wqs, wks, wvs, wos, ebs = {}, {}, {}, {}, {}
    for g in range(4):
        sl = slice(GDIM * g, GDIM * (g + 1))
        wqs[g] = np.ascontiguousarray(
            (Wq[sl, :] / SCALE).T.reshape(8, 128, GDIM)).astype(BF16)
        wks[g] = np.ascontiguousarray(Wk[sl, :].T.reshape(8, 128, GDIM)).astype(BF16)
        wvs[g] = np.ascontiguousarray(Wv[sl, :].T.reshape(8, 128, GDIM)).astype(BF16)
        wos[g] = np.ascontiguousarray(Wo_w[:, sl].T.reshape(2, 128, D)).astype(BF16)
        ebs[g] = _pack_ebias(rel_pos_bias[0, HPC * g:HPC * (g + 1)])
    in_maps = []
    for c in range(N_CORES):
        b, g = c // 4, c % 4
        in_maps.append({
            "qT": xT[("q", b)], "kT": xT[("k", b)], "vT": xT[("v", b)],
            "wq": wqs[g], "wk": wks[g], "wv": wvs[g], "wo": wos[g],
            "eb": ebs[g], "cosT": cosT, "ssinT": ssinT,
        })
    return in_maps


def _run(query, key, value, rel_pos_bias, Wq, Wk, Wv, Wo_w, Wo_b, trace=False,
         **trace_kwargs):
    nc = _get_program()
    in_maps = _prep_inputs(query, key, value, rel_pos_bias, Wq, Wk, Wv, Wo_w)
    res = run_bass_kernel_spmd(nc, in_maps, core_ids=list(range(N_CORES)),
                               trace=trace, **trace_kwargs)
    out = np.empty((B, S, D), dtype=np.float32)
    for b in range(B):
        acc = res.results[4 * b]["out"].astype(np.float32)
        for g in range(1, 4):
            acc = acc + res.results[4 * b + g]["out"].astype(np.float32)
        out[b] = acc + Wo_b[None, :]
    return out, res


def _cpu_fallback(query, key, value, mask, rel_pos_bias, Wq, Wk, Wv, Wo_w, Wo_b):
    def rope_np(x):
        half = DK // 2
        inv_freq = 1.0 / (10000.0 ** (np.arange(half, dtype=np.float32) / half))
        ang = np.arange(S, dtype=np.float32)[:, None] * inv_freq[None, :]
        cos = np.concatenate([np.cos(ang), np.cos(ang)], axis=-1)[None, None]
        sin = np.concatenate([np.sin(ang), np.sin(ang)], axis=-1)[None, None]
        x1, x2 = x[..., :half], x[..., half:]
        rot = np.concatenate([-x2, x1], axis=-1)
        return x * cos + rot * sin

    q = np.einsum('bsd,ed->bse', query, Wq).reshape(B, S, H, DK).transpose(0, 2, 1, 3)
    k = np.einsum('bsd,ed->bse', key, Wk).reshape(B, S, H, DK).transpose(0, 2, 1, 3)
    v = np.einsum('bsd,ed->bse', value, Wv).reshape(B, S, H, DK).transpose(0, 2, 1, 3)
    q, k = rope_np(q), rope_np(k)
    sc = np.einsum('bhqd,bhkd->bhqk', q, k) / SCALE + rel_pos_bias
    sc = np.where(mask, sc, -np.inf)
    sc = sc - sc.max(axis=-1, keepdims=True)
    e = np.exp(sc)
    attn = e / e.sum(axis=-1, keepdims=True)
    ctx = np.einsum('bhqk,bhkd->bhqd', attn, v)
    ctx = ctx.transpose(0, 2, 1, 3).reshape(B, S, D)
    return (np.einsum('bsd,ed->bse', ctx, Wo_w) + Wo_b).astype(np.float32)


def kernel(query, key, value, mask, rel_pos_bias, Wq, Wk, Wv, Wo_w, Wo_b):
    query = np.asarray(query, dtype=np.float32)
    key = np.asarray(key, dtype=np.float32)
    value = np.asarray(value, dtype=np.float32)
    mask = np.asarray(mask)
    rel_pos_bias = np.asarray(rel_pos_bias, dtype=np.float32)
    Wq = np.asarray(Wq, dtype=np.float32)
    Wk = np.asarray(Wk, dtype=np.float32)
    Wv = np.asarray(Wv, dtype=np.float32)
    Wo_w = np.asarray(Wo_w, dtype=np.float32)
    Wo_b = np.asarray(Wo_b, dtype=np.float32)

    if not np.array_equal(mask.reshape(S, S),
                          np.tril(np.ones((S, S), dtype=bool))):
        return _cpu_fallback(query, key, value, mask, rel_pos_bias,
                             Wq, Wk, Wv, Wo_w, Wo_b)

    out, _ = _run(query, key, value, rel_pos_bias, Wq, Wk, Wv, Wo_w, Wo_b)
    return out

